# revision 30
# baseline (speedup 1.0000x reference)
"""Blockdiag butterfly (Monarch) linear on 8 TRN2 NeuronCores.

Math (see reference): x:[B,4096] f32, w1:[4,192,1024], w2:[4,1024,192], bias:[4096]
  stage1: out1[b,k,q] = sum_p x[b, k*1024+p] * w1[k,q,p]          (q = l*48+j)
  block transpose: out1t[b,l,r] = out1[b,k,l*48+j], r = k*48+j
  stage2: out[b, l*1024+s] = sum_r out1t[b,l,r] * w2[l,s,r] + bias

Sharding: pure data-parallel over the batch dim (2048 tokens/core),
weights replicated.  x is transposed host-side (8KB-contiguous per
(k, bt, partition)) so the device never transposes; both matmul stages
contract over the partition dim: stage 1 is weight-stationary
producing psum[q, b], exactly the [r, b] orientation stage 2 needs as
its stationary operand.

All matmul operands are bf16 (PSUM accumulates fp32): fp32 matmuls
cost 4 PE cycles/row on TRN2 vs 1 for bf16, and bf16 halves HBM
traffic.  End-to-end error vs the f32 reference is ~4e-3 relative,
inside the 2e-2 gate.

The per-block width 48 is zero-padded to 64 host-side so the
intermediate lives in 64-row partition groups (engine SBUF access
patterns may only start at partitions {0,32,64,96}) and the
stage1->stage2 block transpose is aligned [64, bt] psum->SBUF copies.
BIAS_IN_MM plants bias in w2t row 255 + a constant-1.0 row in out1t,
making the psum evacuation a plain copy, split ACT/DVE/Pool.

For timing, reps>1 builds a hardware loop (tc.For_i, `unroll` bodies
per iteration) so one NEFF launch executes the body `reps` times
on-device; the back-edge drain+barrier (~24us) amortizes over unroll.
"""

import numpy as np

NB1, NB2, B1 = 4, 4, 48
B1P = 64
IN_F, OUT_F = 4096, 4096
BATCH = 16384
N_CORES = 8
B_LOCAL = BATCH // N_CORES
P = 128
NQP = NB2 * B1P              # 256
NRP = NB1 * B1P              # 256
PC = IN_F // NB1 // P        # 8
BT = 512
NBT = B_LOCAL // BT          # 4
S = OUT_F // NB2             # 1024

_CACHE = {}


def _np_bf16():
    import ml_dtypes
    return np.dtype(ml_dtypes.bfloat16)


def _emit(nc, xt, w1t, w2t, bias, out, reps=1, variant="full", unroll=1,
          s1order="qcmajor", s2order="interleaved"):
    import concourse.mybir as mybir
    import concourse.tile as tile

    f32 = mybir.dt.float32
    bf16 = mybir.dt.bfloat16

    w1_v = w1t.rearrange("k (pc pi) q -> pi k pc q", pc=PC, pi=P)
    w2_v = w2t.rearrange("l (rc ri) s -> ri l rc s", rc=NRP // P, ri=P)

    with tile.TileContext(nc) as tc:
        with (
            tc.tile_pool(name="consts", bufs=1) as consts,
            tc.tile_pool(name="xin", bufs=4) as xin,
            tc.tile_pool(name="mid", bufs=3) as mid,
            tc.tile_pool(name="outp", bufs=6) as outp,
            tc.tile_pool(name="ps1", bufs=2, space="PSUM") as ps1,
            tc.tile_pool(name="ps2", bufs=4, space="PSUM") as ps2,
        ):
            w1_sb = consts.tile([P, NB1, PC, NQP], bf16)
            nc.sync.dma_start(w1_sb[:], w1_v)
            w2_sb = consts.tile([P, NB2, NRP // P, S], bf16)
            nc.sync.dma_start(w2_sb[:], w2_v)
            # keep the bias input alive so the NEFF keeps the tensor
            bias_sb = consts.tile([1, OUT_F], f32)
            nc.sync.dma_start(bias_sb[:], bias[None, :])

            # stage-1 evac engines, round-robin ACT/DVE (Pool cannot
            # read PSUM through walrus)
            evac1 = [nc.scalar.copy,
                     lambda o, i: nc.vector.tensor_copy(out=o, in_=i)]

            def stage1(bt):
                o1 = None
                if variant != "dmaonly":
                    o1 = [
                        mid.tile([P, NB2, BT], bf16, tag="o1a", name="o1a"),
                        mid.tile([P, NB2, BT], bf16, tag="o1b", name="o1b"),
                    ]
                ei = 0
                for k in range(NB1):
                    xk = xin.tile([P, PC, BT], bf16, tag="xk", name="xk")
                    if variant != "nodma":
                        nc.sync.dma_start(xk[:], xt[k, bt])
                    else:
                        nc.gpsimd.memset(xk[:, 0, 0:8], 0.5)
                    if variant == "dmaonly":
                        continue
                    pq = ps1.tile([P, 2, BT], f32, tag="pq", name="pq")
                    if s1order == "qcmajor":
                        order = [(qc, pc) for qc in range(2) for pc in range(PC)]
                    else:
                        order = [(qc, pc) for pc in range(PC) for qc in range(2)]
                    for qc, pc in order:
                        nc.tensor.matmul(
                            pq[:, qc, :],
                            w1_sb[:, k, pc, qc * P:(qc + 1) * P],
                            xk[:, pc, :],
                            start=(pc == 0), stop=(pc == PC - 1),
                        )
                    if k == NB1 - 1:
                        for l in range(NB2):
                            nc.gpsimd.memset(o1[1][96:128, l, :], 1.0)
                        for l in range(NB2):
                            evac1[ei % 2](
                                o1[1][64:64 + B1, l, :],
                                pq[(l % 2) * B1P:(l % 2) * B1P + B1, l // 2, :])
                            ei += 1
                    else:
                        for l in range(NB2):
                            evac1[ei % 2](
                                o1[k // 2][(k % 2) * B1P:(k % 2 + 1) * B1P, l, :],
                                pq[(l % 2) * B1P:(l % 2 + 1) * B1P, l // 2, :])
                            ei += 1
                return o1

            def stage2(bt, o1):
                for bi in range(BT // P):
                    b0 = bt * BT + bi * P
                    bloc = slice(bi * P, (bi + 1) * P)
                    for lp in range(0, NB2, 2):
                        ob = outp.tile([P, 2, S], bf16, tag="ob", name="ob")
                        if variant == "dmaonly":
                            nc.gpsimd.memset(ob[:, 0, 0:8], 0.0)
                            nc.sync.dma_start(
                                out[b0:b0 + P, lp * S:(lp + 2) * S], ob[:])
                            continue
                        pss = {}
                        for sh in range(S // 512):
                            ssl = slice(sh * 512, (sh + 1) * 512)
                            if s2order == "interleaved":
                                for l in (lp, lp + 1):
                                    ps = ps2.tile([P, 512], f32, tag="ps2",
                                                  name="ps2")
                                    pss[(l, sh)] = ps
                                    nc.tensor.matmul(
                                        ps[:], o1[0][:, l, bloc],
                                        w2_sb[:, l, 0, ssl],
                                        start=True, stop=False,
                                    )
                                for l in (lp, lp + 1):
                                    nc.tensor.matmul(
                                        pss[(l, sh)][:], o1[1][:, l, bloc],
                                        w2_sb[:, l, 1, ssl],
                                        start=False, stop=True,
                                    )
                            else:
                                for l in (lp, lp + 1):
                                    ps = ps2.tile([P, 512], f32, tag="ps2",
                                                  name="ps2")
                                    pss[(l, sh)] = ps
                                    nc.tensor.matmul(
                                        ps[:], o1[0][:, l, bloc],
                                        w2_sb[:, l, 0, ssl],
                                        start=True, stop=False,
                                    )
                                    nc.tensor.matmul(
                                        ps[:], o1[1][:, l, bloc],
                                        w2_sb[:, l, 1, ssl],
                                        start=False, stop=True,
                                    )
                        for l in (lp, lp + 1):
                            for sh in range(S // 512):
                                ssl = slice(sh * 512, (sh + 1) * 512)
                                if l % 2 == 1:
                                    nc.scalar.copy(ob[:, 1, ssl], pss[(l, sh)][:])
                                else:
                                    nc.vector.tensor_copy(
                                        out=ob[:, 0, ssl], in_=pss[(l, sh)][:])
                        if variant != "nodma":
                            nc.sync.dma_start(
                                out[b0:b0 + P, lp * S:(lp + 2) * S], ob[:])

            # Software-pipeline the two stages across bt within each
            # loop body: stage2(bt) runs after stage1(bt+1), so its o1
            # reads never wait on freshly-issued evacuations.  No tile
            # crosses the For_i back edge.
            def pipelined_body(n_bodies):
                seq = [bt for _ in range(n_bodies) for bt in range(NBT)]
                prev = stage1(seq[0]), seq[0]
                for bt in seq[1:]:
                    o1 = stage1(bt)
                    stage2(prev[1], prev[0])
                    prev = o1, bt
                stage2(prev[1], prev[0])

            if reps == 1:
                pipelined_body(1)
            else:
                assert reps % unroll == 0
                with tc.For_i(0, reps // unroll, 1):
                    pipelined_body(unroll)


def _build(reps=1, variant="full", unroll=1, s1order="qcmajor",
           s2order="interleaved"):
    import concourse.bacc as bacc
    import concourse.mybir as mybir

    # Bacc (not plain Bass): its compile() legalizes semaphore waits
    # (move_matmul_waits_to_ldweights + generate_event_semaphores) --
    # walrus allows at most one sync wait per instruction.
    nc = bacc.Bacc(name=f"bfly_r{reps}_{variant}_u{unroll}_{s1order}_{s2order}")
    bf16 = mybir.dt.bfloat16
    xt = nc.dram_tensor("xt", [NB1, NBT, P, PC, BT], bf16, kind="ExternalInput")
    w1t = nc.dram_tensor("w1t", [NB1, IN_F // NB1, NQP], bf16, kind="ExternalInput")
    w2t = nc.dram_tensor("w2t", [NB2, NRP, S], bf16, kind="ExternalInput")
    bias = nc.dram_tensor("bias", [OUT_F], mybir.dt.float32, kind="ExternalInput")
    out = nc.dram_tensor("out", [B_LOCAL, OUT_F], bf16, kind="ExternalOutput")
    _emit(nc, xt[:], w1t[:], w2t[:], bias[:], out[:], reps=reps,
          variant=variant, unroll=unroll, s1order=s1order, s2order=s2order)
    nc.compile()
    return nc


def get_nc(reps=1, variant="full", unroll=1, s1order="qcmajor",
           s2order="interleaved"):
    key = ("nc", reps, variant, unroll, s1order, s2order)
    if key not in _CACHE:
        _CACHE[key] = _build(reps, variant, unroll, s1order=s1order,
                             s2order=s2order)
    return _CACHE[key]


def prep_weights(w1_bfly, w2_bfly, bias):
    """Pad the per-block width 48 -> 64, transpose for the device
    layout, cast to bf16; plant bias in w2t's last padding row."""
    bf16 = _np_bf16()
    w1t = np.zeros((NB1, IN_F // NB1, NQP), dtype=bf16)
    w1t_v = w1t.reshape(NB1, IN_F // NB1, NB2, B1P)
    w1t_v[:, :, :, :B1] = (
        np.asarray(w1_bfly, np.float32).transpose(0, 2, 1)
        .reshape(NB1, IN_F // NB1, NB2, B1).astype(bf16)
    )
    w2t = np.zeros((NB2, NRP, S), dtype=bf16)
    w2t_v = w2t.reshape(NB2, NB1, B1P, S)
    w2t_v[:, :, :B1, :] = (
        np.asarray(w2_bfly, np.float32).transpose(0, 2, 1)
        .reshape(NB2, NB1, B1, S).astype(bf16)
    )
    w2t[:, NRP - 1, :] = (
        np.asarray(bias, np.float32).reshape(NB2, S).astype(bf16))
    return w1t, w2t


def _prep_inputs(x, w1_bfly, w2_bfly, bias):
    bf16 = _np_bf16()
    bias = np.ascontiguousarray(np.asarray(bias, np.float32))
    w1t, w2t = prep_weights(w1_bfly, w2_bfly, bias)
    x = np.asarray(x, np.float32).astype(bf16)
    in_maps = []
    for c in range(N_CORES):
        xc = x[c * B_LOCAL:(c + 1) * B_LOCAL]
        # [bt, b, k, pc, pi] -> [k, bt, pi, pc, b]: 8KB-contiguous per
        # (k, bt, partition) for single-descriptor-per-partition DMA
        xs = np.ascontiguousarray(
            xc.reshape(NBT, BT, NB1, PC, P).transpose(2, 0, 4, 3, 1))
        in_maps.append({"xt": xs, "w1t": w1t, "w2t": w2t, "bias": bias})
    return in_maps


def postprocess(core_result):
    """Per-core device outputs -> [B_LOCAL, OUT_F] f32."""
    return np.asarray(core_result["out"]).astype(np.float32)


def kernel(x, w1_bfly, w2_bfly, bias):
    from concourse.bass_utils import run_bass_kernel_spmd

    nc = get_nc()
    in_maps = _prep_inputs(np.asarray(x), np.asarray(w1_bfly),
                           np.asarray(w2_bfly), np.asarray(bias))
    res = run_bass_kernel_spmd(nc, in_maps, list(range(N_CORES)), trace=False)
    return np.concatenate(
        [postprocess(res.results[c]) for c in range(N_CORES)], axis=0)


# revision 31
# speedup vs baseline: 1.0295x; 1.0295x over previous
"""Blockdiag butterfly (Monarch) linear on 8 TRN2 NeuronCores.

Math (see reference): x:[B,4096] f32, w1:[4,192,1024], w2:[4,1024,192], bias:[4096]
  stage1: out1[b,k,q] = sum_p x[b, k*1024+p] * w1[k,q,p]          (q = l*48+j)
  block transpose: out1t[b,l,r] = out1[b,k,l*48+j], r = k*48+j
  stage2: out[b, l*1024+s] = sum_r out1t[b,l,r] * w2[l,s,r] + bias

Sharding: pure data-parallel over the batch dim (2048 tokens/core),
weights replicated.  x is transposed host-side (8KB-contiguous per
(k, bt, partition)) so the device never transposes; both matmul stages
contract over the partition dim: stage 1 is weight-stationary
producing psum[q, b], exactly the [r, b] orientation stage 2 needs as
its stationary operand.

All matmul operands are bf16 (PSUM accumulates fp32): fp32 matmuls
cost 4 PE cycles/row on TRN2 vs 1 for bf16, and bf16 halves HBM
traffic.  End-to-end error vs the f32 reference is ~4e-3 relative,
inside the 2e-2 gate.

The per-block width 48 is zero-padded to 64 host-side so the
intermediate lives in 64-row partition groups (engine SBUF access
patterns may only start at partitions {0,32,64,96}) and the
stage1->stage2 block transpose is aligned [64, bt] psum->SBUF copies.
BIAS_IN_MM plants bias in w2t row 255 + a constant-1.0 row in out1t,
making the psum evacuation a plain copy, split ACT/DVE/Pool.

For timing, reps>1 builds a hardware loop (tc.For_i, `unroll` bodies
per iteration) so one NEFF launch executes the body `reps` times
on-device; the back-edge drain+barrier (~24us) amortizes over unroll.
"""

import numpy as np

NB1, NB2, B1 = 4, 4, 48
B1P = 64
IN_F, OUT_F = 4096, 4096
BATCH = 16384
N_CORES = 8
B_LOCAL = BATCH // N_CORES
P = 128
NQP = NB2 * B1P              # 256
NRP = NB1 * B1P              # 256
PC = IN_F // NB1 // P        # 8
BT = 512
NBT = B_LOCAL // BT          # 4
S = OUT_F // NB2             # 1024

_CACHE = {}


def _np_bf16():
    import ml_dtypes
    return np.dtype(ml_dtypes.bfloat16)


def _emit(nc, xt, w1t, w2t, bias, out, reps=1, variant="full", unroll=1,
          s1order="qcmajor", s2order="interleaved"):
    import concourse.mybir as mybir
    import concourse.tile as tile

    f32 = mybir.dt.float32
    bf16 = mybir.dt.bfloat16

    w1_v = w1t.rearrange("k (pc pi) q -> pi k pc q", pc=PC, pi=P)
    w2_v = w2t.rearrange("l (rc ri) s -> ri l rc s", rc=NRP // P, ri=P)

    with tile.TileContext(nc) as tc:
        with (
            tc.tile_pool(name="consts", bufs=1) as consts,
            tc.tile_pool(name="xin", bufs=6) as xin,
            tc.tile_pool(name="mid", bufs=3) as mid,
            tc.tile_pool(name="outp", bufs=8) as outp,
            tc.tile_pool(name="ps1", bufs=3, space="PSUM") as ps1,
            tc.tile_pool(name="ps2", bufs=5, space="PSUM") as ps2,
        ):
            w1_sb = consts.tile([P, NB1, PC, NQP], bf16)
            nc.sync.dma_start(w1_sb[:], w1_v)
            w2_sb = consts.tile([P, NB2, NRP // P, S], bf16)
            nc.sync.dma_start(w2_sb[:], w2_v)
            # keep the bias input alive so the NEFF keeps the tensor
            bias_sb = consts.tile([1, OUT_F], f32)
            nc.sync.dma_start(bias_sb[:], bias[None, :])

            # stage-1 evac engines, round-robin ACT/DVE (Pool cannot
            # read PSUM through walrus)
            evac1 = [nc.scalar.copy,
                     lambda o, i: nc.vector.tensor_copy(out=o, in_=i)]

            def stage1(bt):
                o1 = None
                if variant != "dmaonly":
                    o1 = [
                        mid.tile([P, NB2, BT], bf16, tag="o1a", name="o1a"),
                        mid.tile([P, NB2, BT], bf16, tag="o1b", name="o1b"),
                    ]
                ei = 0
                for k in range(NB1):
                    xk = xin.tile([P, PC, BT], bf16, tag="xk", name="xk")
                    if variant != "nodma":
                        nc.sync.dma_start(xk[:], xt[k, bt])
                    else:
                        nc.gpsimd.memset(xk[:, 0, 0:8], 0.5)
                    if variant == "dmaonly":
                        continue
                    pq = [ps1.tile([P, BT], f32, tag="pq", name="pq"),
                          ps1.tile([P, BT], f32, tag="pq", name="pq")]

                    def evac(l):
                        nonlocal ei
                        src = pq[l // 2]
                        if k == NB1 - 1:
                            evac1[ei % 2](
                                o1[1][64:64 + B1, l, :],
                                src[(l % 2) * B1P:(l % 2) * B1P + B1, :])
                        else:
                            evac1[ei % 2](
                                o1[k // 2][(k % 2) * B1P:(k % 2 + 1) * B1P, l, :],
                                src[(l % 2) * B1P:(l % 2 + 1) * B1P, :])
                        ei += 1

                    if k == NB1 - 1:
                        for l in range(NB2):
                            nc.gpsimd.memset(o1[1][96:128, l, :], 1.0)
                    for qc in range(2):
                        for pc in range(PC):
                            nc.tensor.matmul(
                                pq[qc][:],
                                w1_sb[:, k, pc, qc * P:(qc + 1) * P],
                                xk[:, pc, :],
                                start=(pc == 0), stop=(pc == PC - 1),
                            )
                        # qc0's l-blocks evacuate while qc1 accumulates
                        for l in (2 * qc, 2 * qc + 1):
                            evac(l)
                return o1

            def stage2(bt, o1):
                for bi in range(BT // P):
                    b0 = bt * BT + bi * P
                    bloc = slice(bi * P, (bi + 1) * P)
                    for lp in range(0, NB2, 2):
                        ob = outp.tile([P, 2, S], bf16, tag="ob", name="ob")
                        if variant == "dmaonly":
                            nc.gpsimd.memset(ob[:, 0, 0:8], 0.0)
                            nc.sync.dma_start(
                                out[b0:b0 + P, lp * S:(lp + 2) * S], ob[:])
                            continue
                        pss = {}
                        for sh in range(S // 512):
                            ssl = slice(sh * 512, (sh + 1) * 512)
                            if s2order == "interleaved":
                                for l in (lp, lp + 1):
                                    ps = ps2.tile([P, 512], f32, tag="ps2",
                                                  name="ps2")
                                    pss[(l, sh)] = ps
                                    nc.tensor.matmul(
                                        ps[:], o1[0][:, l, bloc],
                                        w2_sb[:, l, 0, ssl],
                                        start=True, stop=False,
                                    )
                                for l in (lp, lp + 1):
                                    nc.tensor.matmul(
                                        pss[(l, sh)][:], o1[1][:, l, bloc],
                                        w2_sb[:, l, 1, ssl],
                                        start=False, stop=True,
                                    )
                            else:
                                for l in (lp, lp + 1):
                                    ps = ps2.tile([P, 512], f32, tag="ps2",
                                                  name="ps2")
                                    pss[(l, sh)] = ps
                                    nc.tensor.matmul(
                                        ps[:], o1[0][:, l, bloc],
                                        w2_sb[:, l, 0, ssl],
                                        start=True, stop=False,
                                    )
                                    nc.tensor.matmul(
                                        ps[:], o1[1][:, l, bloc],
                                        w2_sb[:, l, 1, ssl],
                                        start=False, stop=True,
                                    )
                        for l in (lp, lp + 1):
                            for sh in range(S // 512):
                                ssl = slice(sh * 512, (sh + 1) * 512)
                                if l % 2 == 1:
                                    nc.scalar.copy(ob[:, 1, ssl], pss[(l, sh)][:])
                                else:
                                    nc.vector.tensor_copy(
                                        out=ob[:, 0, ssl], in_=pss[(l, sh)][:])
                        if variant != "nodma":
                            nc.sync.dma_start(
                                out[b0:b0 + P, lp * S:(lp + 2) * S], ob[:])

            # Software-pipeline the two stages across bt within each
            # loop body: stage2(bt) runs after stage1(bt+1), so its o1
            # reads never wait on freshly-issued evacuations.  No tile
            # crosses the For_i back edge.
            def pipelined_body(n_bodies):
                seq = [bt for _ in range(n_bodies) for bt in range(NBT)]
                prev = stage1(seq[0]), seq[0]
                for bt in seq[1:]:
                    o1 = stage1(bt)
                    stage2(prev[1], prev[0])
                    prev = o1, bt
                stage2(prev[1], prev[0])

            if reps == 1:
                pipelined_body(1)
            else:
                assert reps % unroll == 0
                with tc.For_i(0, reps // unroll, 1):
                    pipelined_body(unroll)


def _build(reps=1, variant="full", unroll=1, s1order="qcmajor",
           s2order="interleaved"):
    import concourse.bacc as bacc
    import concourse.mybir as mybir

    # Bacc (not plain Bass): its compile() legalizes semaphore waits
    # (move_matmul_waits_to_ldweights + generate_event_semaphores) --
    # walrus allows at most one sync wait per instruction.
    nc = bacc.Bacc(name=f"bfly_r{reps}_{variant}_u{unroll}_{s1order}_{s2order}")
    bf16 = mybir.dt.bfloat16
    xt = nc.dram_tensor("xt", [NB1, NBT, P, PC, BT], bf16, kind="ExternalInput")
    w1t = nc.dram_tensor("w1t", [NB1, IN_F // NB1, NQP], bf16, kind="ExternalInput")
    w2t = nc.dram_tensor("w2t", [NB2, NRP, S], bf16, kind="ExternalInput")
    bias = nc.dram_tensor("bias", [OUT_F], mybir.dt.float32, kind="ExternalInput")
    out = nc.dram_tensor("out", [B_LOCAL, OUT_F], bf16, kind="ExternalOutput")
    _emit(nc, xt[:], w1t[:], w2t[:], bias[:], out[:], reps=reps,
          variant=variant, unroll=unroll, s1order=s1order, s2order=s2order)
    nc.compile()
    return nc


def get_nc(reps=1, variant="full", unroll=1, s1order="qcmajor",
           s2order="interleaved"):
    key = ("nc", reps, variant, unroll, s1order, s2order)
    if key not in _CACHE:
        _CACHE[key] = _build(reps, variant, unroll, s1order=s1order,
                             s2order=s2order)
    return _CACHE[key]


def prep_weights(w1_bfly, w2_bfly, bias):
    """Pad the per-block width 48 -> 64, transpose for the device
    layout, cast to bf16; plant bias in w2t's last padding row."""
    bf16 = _np_bf16()
    w1t = np.zeros((NB1, IN_F // NB1, NQP), dtype=bf16)
    w1t_v = w1t.reshape(NB1, IN_F // NB1, NB2, B1P)
    w1t_v[:, :, :, :B1] = (
        np.asarray(w1_bfly, np.float32).transpose(0, 2, 1)
        .reshape(NB1, IN_F // NB1, NB2, B1).astype(bf16)
    )
    w2t = np.zeros((NB2, NRP, S), dtype=bf16)
    w2t_v = w2t.reshape(NB2, NB1, B1P, S)
    w2t_v[:, :, :B1, :] = (
        np.asarray(w2_bfly, np.float32).transpose(0, 2, 1)
        .reshape(NB2, NB1, B1, S).astype(bf16)
    )
    w2t[:, NRP - 1, :] = (
        np.asarray(bias, np.float32).reshape(NB2, S).astype(bf16))
    return w1t, w2t


def _prep_inputs(x, w1_bfly, w2_bfly, bias):
    bf16 = _np_bf16()
    bias = np.ascontiguousarray(np.asarray(bias, np.float32))
    w1t, w2t = prep_weights(w1_bfly, w2_bfly, bias)
    x = np.asarray(x, np.float32).astype(bf16)
    in_maps = []
    for c in range(N_CORES):
        xc = x[c * B_LOCAL:(c + 1) * B_LOCAL]
        # [bt, b, k, pc, pi] -> [k, bt, pi, pc, b]: 8KB-contiguous per
        # (k, bt, partition) for single-descriptor-per-partition DMA
        xs = np.ascontiguousarray(
            xc.reshape(NBT, BT, NB1, PC, P).transpose(2, 0, 4, 3, 1))
        in_maps.append({"xt": xs, "w1t": w1t, "w2t": w2t, "bias": bias})
    return in_maps


def postprocess(core_result):
    """Per-core device outputs -> [B_LOCAL, OUT_F] f32."""
    return np.asarray(core_result["out"]).astype(np.float32)


def kernel(x, w1_bfly, w2_bfly, bias):
    from concourse.bass_utils import run_bass_kernel_spmd

    nc = get_nc()
    in_maps = _prep_inputs(np.asarray(x), np.asarray(w1_bfly),
                           np.asarray(w2_bfly), np.asarray(bias))
    res = run_bass_kernel_spmd(nc, in_maps, list(range(N_CORES)), trace=False)
    return np.concatenate(
        [postprocess(res.results[c]) for c in range(N_CORES)], axis=0)
